# revision 47
# baseline (speedup 1.0000x reference)
"""Trainium2 Bass kernel for nn_DeformSpaceAttentionv5 (deformable 3x3 unfold
+ per-channel max + two 1x1 convs + channel-norm dot product).

Contract: kernel(**inputs) takes the FULL inputs (x [4,256,128,128] f32,
offset [4,18,128,128] f32, w0/w1 [256,256] f32, b0/b1 [256] f32) and returns
the FULL output [4,1,128,128] f32.

Strategy (pure data parallel over 8 NeuronCores): core = (batch, H-half).
The padded image is stored in DRAM twice in a "row-pair" channel-last layout
(pairs starting at even rows and at odd rows), so that ONE gather descriptor
fetches the full 2x2 bilinear patch (2 rows x 2 cols x 256 ch = 1024 fp16
contiguous).  SWDGE dma_gather brings 9 such patches per position.  The
bilinear interp uses the 4-corner factored form with host-precomputed corner
products w00..w11: per sample two independent ACT->DVE chains
(ACT: a=A*w00, c=C*w10 via activation-with-scale; DVE: s1=B*w01+a,
s2=D*w11+c via scalar_tensor_tensor), then one wide 9k add s1+s2 and a
4-op max tree on DVE, all in position-major layout.  PE does q transposes
and the two 1x1 convs (q^T w0^T / x^T w1^T with an extra channel-sum
column), followed by a fused normalized-correlation epilogue (ACT
square-accumulate, DVE product-accumulate, final combine once per core).
"""

import numpy as np

B, C, H, W = 4, 256, 128, 128
PAD = 8
Hp, Wp = H + 2 * PAD, W + 2 * PAD
ROWS = 64            # rows per core (H split in 2)
N = ROWS * W         # positions per core
BLK = 128            # positions per block (= one row)
NBLK = N // BLK      # 64
GRP = 2              # blocks per gather group
NG = NBLK // GRP     # 32
NIDX = GRP * 9 * BLK  # gather indices per group (2 blk * 9 k * 128 pos)
EPS = 1e-5

# row-pair layout: A = pairs (0,1),(2,3),...,(142,143); B = (1,2),...,(141,142)
NPA = (Hp // 2) * Wp          # 72*144 = 10368 elements (each 512 fp16)
NPB = (Hp // 2 - 1) * Wp      # 71*144 = 10224
NPT = NPA + NPB               # 20592  (< int16 max)

_NC_CACHE = {}


def _build_nc(has_bias: bool, n_groups: int = NG):
    import concourse.bacc as bacc
    import concourse.bass as bass
    import concourse.tile as tile
    import concourse.mybir as mybir
    from concourse import library_config

    f16 = mybir.dt.float16
    f32 = mybir.dt.float32
    i16 = mybir.dt.int16
    Alu = mybir.AluOpType
    Act = mybir.ActivationFunctionType

    nc = bacc.Bacc("TRN2", target_bir_lowering=False, debug=False, num_devices=8)

    xt = nc.dram_tensor("xt", [(NPT + 1) * 512], f16, kind="ExternalInput")
    xk = nc.dram_tensor("xk", [2, 128, N], f16, kind="ExternalInput")
    idx = nc.dram_tensor("idx", [n_groups, 128, NIDX // 16], i16, kind="ExternalInput")
    w4 = nc.dram_tensor("w4", [n_groups, 128, GRP, 18], f16, kind="ExternalInput")
    w4f = nc.dram_tensor("w4f", [n_groups, 128, GRP, 18], f32, kind="ExternalInput")
    w0t = nc.dram_tensor("w0t", [2, 128, 257], f16, kind="ExternalInput")
    w1t = nc.dram_tensor("w1t", [2, 128, 257], f16, kind="ExternalInput")
    idmat = nc.dram_tensor("idmat", [128, 128], f16, kind="ExternalInput")
    if has_bias:
        qb = nc.dram_tensor("qb", [128, 257], f32, kind="ExternalInput")
        kb = nc.dram_tensor("kb", [128, 257], f32, kind="ExternalInput")
    nblk_t = n_groups * GRP
    o = nc.dram_tensor("o", [128, nblk_t], f32, kind="ExternalOutput")

    # overlapping-window gather view: element j = xt[j*512 : j*512+1024]
    xt_view = bass.AP(tensor=xt[:].tensor, offset=0, ap=[[512, NPT], [1, 1024]])

    with tile.TileContext(nc) as tc:
        import contextlib

        with contextlib.ExitStack() as ctx:
            consts = ctx.enter_context(tc.tile_pool(name="consts", bufs=1))
            gpool = ctx.enter_context(tc.tile_pool(name="gath", bufs=3))
            iopool = ctx.enter_context(tc.tile_pool(name="io", bufs=3))
            work = ctx.enter_context(tc.tile_pool(name="work", bufs=3))
            kpool = ctx.enter_context(tc.tile_pool(name="kp", bufs=6))
            pspool = ctx.enter_context(tc.tile_pool(name="ps", bufs=2, space="PSUM"))

            # first group's gather indices go out before anything else so the
            # first dma_gather can start as early as possible (scalar HWDGE
            # queue, parallel to the sync-queue const loads)
            idx0_t = iopool.tile([128, NIDX // 16], i16, tag="idx")
            nc.sync.dma_start(out=idx0_t, in_=idx[0])

            # constants
            w0t_sb = consts.tile([128, 2, 257], f16)
            nc.sync.dma_start(out=w0t_sb, in_=w0t[:, :, :].rearrange("t p o -> p t o"))
            w1t_sb = consts.tile([128, 2, 257], f16)
            nc.sync.dma_start(out=w1t_sb, in_=w1t[:, :, :].rearrange("t p o -> p t o"))
            ident = consts.tile([128, 128], f16)
            nc.sync.dma_start(out=ident, in_=idmat[:, :])
            if has_bias:
                qb_sb = consts.tile([128, 257], f32)
                nc.sync.dma_start(out=qb_sb, in_=qb[:, :])
                kb_sb = consts.tile([128, 257], f32)
                nc.sync.dma_start(out=kb_sb, in_=kb[:, :])

            # per-block scalar accumulators [128 pos, NBLK]
            sqs = consts.tile([128, nblk_t], f32, tag="sqs")
            sks = consts.tile([128, nblk_t], f32, tag="sks")
            sqks = consts.tile([128, nblk_t], f32, tag="sqks")
            sQs = consts.tile([128, nblk_t], f32, tag="sQs")
            sKs = consts.tile([128, nblk_t], f32, tag="sKs")

            nc.gpsimd.load_library(library_config.mlp)

            for g in range(n_groups):
                if g == 0:
                    idx_t = idx0_t
                else:
                    idx_t = iopool.tile([128, NIDX // 16], i16, tag="idx")
                    nc.sync.dma_start(out=idx_t, in_=idx[g])
                w4_t = iopool.tile([128, GRP, 18], f16, tag="w4")
                nc.sync.dma_start(out=w4_t, in_=w4[g])
                w4f_t = iopool.tile([128, GRP, 18], f32, tag="w4f")
                nc.sync.dma_start(out=w4f_t, in_=w4f[g])
                xk_t = iopool.tile([128, 2, GRP * BLK], f16, tag="xk")
                nc.sync.dma_start(
                    out=xk_t, in_=xk[:, :, g * GRP * BLK:(g + 1) * GRP * BLK]
                    .rearrange("t p n -> p t n")
                )
                # gat[p, blk, k, x, r, c]: 2x2 patch (x = column, r = row)
                gat = gpool.tile([128, GRP, 9, 2, 2, 256], f16, tag="gat")
                if g == 0:
                    # split the first gathers so compute starts sooner:
                    # blk0 in three 3-k chunks, blk1 whole
                    for k0 in range(0, 9, 3):
                        nch = 3 * BLK
                        nc.gpsimd.dma_gather(
                            gat[:, 0, k0:k0 + 3].rearrange(
                                "p k x r c -> p k (x r c)"),
                            xt_view,
                            idx_t[:, k0 * (BLK // 16):(k0 + 3) * (BLK // 16)],
                            nch, nch, 1024, elem_step=512,
                            single_packet=False,
                        )
                    nido2 = NIDX // GRP
                    nc.gpsimd.dma_gather(
                        gat[:, 1].rearrange("p k x r c -> p k (x r c)"),
                        xt_view,
                        idx_t[:, (nido2 // 16):],
                        nido2, nido2, 1024, elem_step=512,
                        single_packet=False,
                    )
                else:
                    nc.gpsimd.dma_gather(
                        gat.rearrange("p a k x r c -> p (a k) (x r c)"),
                        xt_view, idx_t, NIDX, NIDX, 1024, elem_step=512,
                        single_packet=False,
                    )

                for blk in range(GRP):
                    nblk = g * GRP + blk
                    s1_all = kpool.tile([128, 9, 256], f16, tag="s1a", bufs=2)
                    s2_all = kpool.tile([128, 9, 256], f16, tag="s2a", bufs=2)
                    for k in range(9):
                        # 2x2 patch corners, each contiguous 256:
                        # A=(x0,y0) B=(x0,y1) C=(x1,y0) D=(x1,y1)
                        A = gat[:, blk, k, 0, 0, :]
                        Bc = gat[:, blk, k, 0, 1, :]
                        Cc = gat[:, blk, k, 1, 0, :]
                        D = gat[:, blk, k, 1, 1, :]
                        # host-precomputed corner products:
                        # w4f (f32, ACT): wa=(1-fx)(1-fy), wc=fx(1-fy)
                        # w4  (f16, DVE): wb=(1-fx)fy,    wd=fx*fy
                        wa = w4f_t[:, blk, k:k + 1]
                        wc = w4f_t[:, blk, 9 + k:10 + k]
                        wb = w4_t[:, blk, k:k + 1]
                        wd = w4_t[:, blk, 9 + k:10 + k]
                        # two independent ACT->DVE chains per k
                        a_t = kpool.tile([128, 256], f16, tag="ta", bufs=8)
                        nc.scalar.activation(a_t, A, Act.Copy, bias=0.0, scale=wa)
                        c_t = kpool.tile([128, 256], f16, tag="tc", bufs=8)
                        nc.scalar.activation(c_t, Cc, Act.Copy, bias=0.0, scale=wc)
                        nc.vector.scalar_tensor_tensor(
                            s1_all[:, k, :], Bc, wb, a_t, Alu.mult, Alu.add
                        )
                        nc.vector.scalar_tensor_tensor(
                            s2_all[:, k, :], D, wd, c_t, Alu.mult, Alu.add
                        )
                    # one wide add (2304 elems), then 9-way max tree on DVE
                    nc.vector.tensor_tensor(s1_all, s1_all, s2_all, Alu.add)
                    nc.vector.tensor_tensor(
                        s1_all[:, 0:4, :], s1_all[:, 0:4, :], s1_all[:, 4:8, :],
                        Alu.max,
                    )
                    nc.vector.tensor_tensor(
                        s1_all[:, 0:2, :], s1_all[:, 0:2, :], s1_all[:, 2:4, :],
                        Alu.max,
                    )
                    nc.vector.tensor_tensor(
                        s1_all[:, 0, :], s1_all[:, 0, :], s1_all[:, 1, :], Alu.max
                    )
                    q_t = work.tile([128, 256], f16, tag="q")
                    nc.vector.tensor_tensor(
                        q_t, s1_all[:, 0, :], s1_all[:, 8, :], Alu.max
                    )

                    # transpose q -> qT (c-major) via PE
                    qt_ps = pspool.tile([128, 2, 128], f16, tag="qt")
                    for t in range(2):
                        nc.tensor.transpose(
                            qt_ps[:, t, :], q_t[:, t * 128:(t + 1) * 128], ident
                        )
                    qt_sb = work.tile([128, 2, 128], f16, tag="qt_sb")
                    nc.vector.tensor_copy(qt_sb, qt_ps)

                    # Q = qT^T @ w0t  -> [128 pos, 257] (col 256 = sum_o Q)
                    Q_ps = pspool.tile([128, 257], f32, tag="Q", bufs=3)
                    for t in range(2):
                        nc.tensor.matmul(
                            Q_ps, qt_sb[:, t, :], w0t_sb[:, t, :],
                            start=(t == 0), stop=(t == 1),
                        )
                    K_ps = pspool.tile([128, 257], f32, tag="K", bufs=3)
                    for t in range(2):
                        nc.tensor.matmul(
                            K_ps, xk_t[:, t, blk * BLK:(blk + 1) * BLK],
                            w1t_sb[:, t, :], start=(t == 0), stop=(t == 1),
                        )
                    if has_bias:
                        nc.vector.tensor_tensor(Q_ps, Q_ps, qb_sb, Alu.add)
                        nc.vector.tensor_tensor(K_ps, K_ps, kb_sb, Alu.add)

                    # epilogue reductions
                    col = slice(nblk, nblk + 1)
                    act_scr = work.tile([128, 256], f16, tag="act_scr")
                    nc.scalar.activation(
                        act_scr, Q_ps[:, 0:256], Act.Square,
                        accum_out=sqs[:, col],
                    )
                    K_sb = work.tile([128, 256], f16, tag="K_sb")
                    nc.scalar.copy(K_sb, K_ps[:, 0:256])
                    nc.scalar.activation(
                        act_scr, K_ps[:, 0:256], Act.Square, accum_out=sks[:, col],
                    )
                    dve_scr = work.tile([128, 256], f16, tag="dve_scr")
                    nc.vector.scalar_tensor_tensor(
                        dve_scr, Q_ps[:, 0:256], 0.0, K_sb, Alu.bypass, Alu.mult,
                        accum_out=sqks[:, col],
                    )
                    nc.vector.tensor_copy(sQs[:, col], Q_ps[:, 256:257])
                    nc.vector.tensor_copy(sKs[:, col], K_ps[:, 256:257])

                if g == n_groups // 2 - 1 or g == n_groups - 1:
                    # combine + store the finished half of the output columns
                    # (first half fires mid-kernel, hiding the serial tail)
                    h = slice(0, nblk_t // 2) if g < n_groups - 1 else slice(
                        nblk_t // 2, nblk_t)
                    nh = nblk_t // 2
                    tmp = consts.tile([128, nh], f32, tag=f"tmp{h.start}")
                    num = consts.tile([128, nh], f32, tag=f"num{h.start}")
                    dq = consts.tile([128, nh], f32, tag=f"dq{h.start}")
                    dk = consts.tile([128, nh], f32, tag=f"dk{h.start}")
                    out_t = consts.tile([128, nh], f32, tag=f"out{h.start}")
                    inv_c = -1.0 / C
                    # num = sqk - sQ*sK/C
                    nc.vector.tensor_tensor(tmp, sQs[:, h], sKs[:, h], Alu.mult)
                    nc.vector.scalar_tensor_tensor(
                        num, tmp, inv_c, sqks[:, h], Alu.mult, Alu.add)
                    # dq = sq - sQ^2/C + eps
                    nc.vector.tensor_tensor(tmp, sQs[:, h], sQs[:, h], Alu.mult)
                    nc.vector.scalar_tensor_tensor(
                        dq, tmp, inv_c, sqs[:, h], Alu.mult, Alu.add)
                    nc.vector.tensor_scalar(dq, dq, EPS, None, Alu.add)
                    nc.vector.tensor_tensor(tmp, sKs[:, h], sKs[:, h], Alu.mult)
                    nc.vector.scalar_tensor_tensor(
                        dk, tmp, inv_c, sks[:, h], Alu.mult, Alu.add)
                    nc.vector.tensor_scalar(dk, dk, EPS, None, Alu.add)
                    # out = num / sqrt(dq*dk)
                    nc.vector.tensor_tensor(tmp, dq, dk, Alu.mult)
                    nc.scalar.activation(tmp, tmp, Act.Sqrt)
                    nc.vector.reciprocal(tmp, tmp)
                    nc.vector.tensor_tensor(out_t, num, tmp, Alu.mult)
                    nc.sync.dma_start(out=o[:, h], in_=out_t)

    nc.compile()
    return nc


def _get_nc(has_bias: bool):
    if has_bias not in _NC_CACHE:
        _NC_CACHE[has_bias] = _build_nc(has_bias)
    return _NC_CACHE[has_bias]


def _build_pair(x_b):
    """Row-pair channel-last layout: A-pairs (even start) then B-pairs (odd),
    with one trailing 512-elem pad element."""
    xp = np.zeros((Hp, Wp, C), np.float16)
    xp[PAD:PAD + H, PAD:PAD + W, :] = x_b.transpose(1, 2, 0)
    pa = xp.reshape(Hp // 2, 2, Wp, C).transpose(0, 2, 1, 3)  # [p, x, r, c]
    pb = xp[1:Hp - 1].reshape(Hp // 2 - 1, 2, Wp, C).transpose(0, 2, 1, 3)
    flat = np.empty(((NPT + 1) * 512,), np.float16)
    flat[:NPA * 512] = pa.reshape(-1)
    flat[NPA * 512:NPT * 512] = pb.reshape(-1)
    flat[NPT * 512:] = 0
    return flat


def _prep_core(x_b, off_b, h0):
    """Host-side shard prep for one core: indices, weights, fp16 layouts."""
    ys, xs = np.meshgrid(
        np.arange(h0, h0 + ROWS), np.arange(W), indexing="ij"
    )
    ys = ys.reshape(-1).astype(np.float32)
    xs = xs.reshape(-1).astype(np.float32)

    idx_all = np.empty((N, 9), np.int32)
    fy_all = np.empty((N, 9), np.float32)
    fx_all = np.empty((N, 9), np.float32)
    for k in range(9):
        kh, kw = k // 3 - 1, k % 3 - 1
        iy = ys.astype(np.int32)
        ix = xs.astype(np.int32)
        py = ys + kh + off_b[2 * k, iy, ix]
        px = xs + kw + off_b[2 * k + 1, iy, ix]
        y0 = np.clip(np.floor(py).astype(np.int32), -PAD, H + PAD - 2)
        x0 = np.clip(np.floor(px).astype(np.int32), -PAD, W + PAD - 2)
        fy_all[:, k] = py - y0
        fx_all[:, k] = px - x0
        yp = y0 + PAD  # [0, 142]
        xpp = x0 + PAD
        even = (yp % 2) == 0
        idx_all[:, k] = np.where(
            even, (yp // 2) * Wp + xpp, NPA + (yp // 2) * Wp + xpp
        )

    # idx tensor [NG, 128, NIDX//16]: slot m = j*128 + p, j = blk*9 + k
    idx_np = np.empty((NG, 128, NIDX // 16), np.int16)
    for g in range(NG):
        slots = np.empty((GRP * 9, BLK), np.int32)
        for blk in range(GRP):
            base = (g * GRP + blk) * BLK
            for k in range(9):
                slots[blk * 9 + k, :] = idx_all[base:base + BLK, k]
        wrapped = slots.reshape(-1).reshape(NIDX // 16, 16).T  # [16, cols]
        idx_np[g] = np.tile(wrapped, (8, 1)).astype(np.int16)

    # corner-product weights: fp16 [NG, 128, GRP, 18] (wb, wd) for DVE,
    # fp32 [.., 18] (wa, wc) for ACT scales
    w4_np = np.empty((NG, 128, GRP, 18), np.float16)
    w4f_np = np.empty((NG, 128, GRP, 18), np.float32)
    fy = fy_all.reshape(NBLK, BLK, 9)
    fx = fx_all.reshape(NBLK, BLK, 9)
    for g in range(NG):
        for blk in range(GRP):
            nb = g * GRP + blk
            w4_np[g, :, blk, 0:9] = (1.0 - fx[nb]) * fy[nb]
            w4_np[g, :, blk, 9:18] = fx[nb] * fy[nb]
            w4f_np[g, :, blk, 0:9] = (1.0 - fx[nb]) * (1.0 - fy[nb])
            w4f_np[g, :, blk, 9:18] = fx[nb] * (1.0 - fy[nb])

    xk_np = np.ascontiguousarray(
        x_b.reshape(2, 128, H, W)[:, :, h0:h0 + ROWS, :].reshape(2, 128, N)
    ).astype(np.float16)
    return idx_np, w4_np, w4f_np, xk_np


def _build_in_maps(x, offset, w0, b0, w1, b1, has_bias):
    w0t_np = np.concatenate([w0.T, w0.sum(0)[:, None]], 1).astype(np.float16)
    w1t_np = np.concatenate([w1.T, w1.sum(0)[:, None]], 1).astype(np.float16)
    w0t_np = np.ascontiguousarray(w0t_np.reshape(2, 128, 257))
    w1t_np = np.ascontiguousarray(w1t_np.reshape(2, 128, 257))

    in_maps = []
    xt_cache = {}
    for core in range(8):
        b, half = core // 2, core % 2
        h0 = ROWS * half
        if b not in xt_cache:
            xt_cache[b] = _build_pair(x[b])
        idx_np, w4_np, w4f_np, xk_np = _prep_core(x[b], offset[b], h0)
        m = {
            "idmat": np.eye(128, dtype=np.float16),
            "xt": xt_cache[b],
            "xk": xk_np,
            "idx": idx_np,
            "w4": w4_np,
            "w4f": w4f_np,
            "w0t": w0t_np,
            "w1t": w1t_np,
        }
        if has_bias:
            qb_np = np.concatenate([b0, [b0.sum()]]).astype(np.float32)
            kb_np = np.concatenate([b1, [b1.sum()]]).astype(np.float32)
            m["qb"] = np.tile(qb_np[None, :], (128, 1))
            m["kb"] = np.tile(kb_np[None, :], (128, 1))
        in_maps.append(m)
    return in_maps


def kernel(x, offset, w0, b0, w1, b1):
    from concourse.bass_utils import run_bass_kernel_spmd

    x = np.asarray(x, np.float32)
    offset = np.asarray(offset, np.float32)
    w0 = np.asarray(w0, np.float32)
    w1 = np.asarray(w1, np.float32)
    b0 = np.asarray(b0, np.float32)
    b1 = np.asarray(b1, np.float32)

    has_bias = bool(np.any(b0)) or bool(np.any(b1))
    nc = _get_nc(has_bias)
    in_maps = _build_in_maps(x, offset, w0, b0, w1, b1, has_bias)

    res = run_bass_kernel_spmd(nc, in_maps, core_ids=list(range(8)))

    out = np.empty((B, 1, H, W), np.float32)
    for core in range(8):
        b, half = core // 2, core % 2
        h0 = ROWS * half
        o = res.results[core]["o"]  # [128 pos(x), 64 rows]
        out[b, 0, h0:h0 + ROWS, :] = o.T
    return out


# revision 48
# speedup vs baseline: 1.0139x; 1.0139x over previous
"""Trainium2 Bass kernel for nn_DeformSpaceAttentionv5 (deformable 3x3 unfold
+ per-channel max + two 1x1 convs + channel-norm dot product).

Contract: kernel(**inputs) takes the FULL inputs (x [4,256,128,128] f32,
offset [4,18,128,128] f32, w0/w1 [256,256] f32, b0/b1 [256] f32) and returns
the FULL output [4,1,128,128] f32.

Strategy (pure data parallel over 8 NeuronCores): core = (batch, H-half).
The padded image is stored in DRAM twice in a "row-pair" channel-last layout
(pairs starting at even rows and at odd rows), so that ONE gather descriptor
fetches the full 2x2 bilinear patch (2 rows x 2 cols x 256 ch = 1024 fp16
contiguous).  SWDGE dma_gather brings 9 such patches per position.  The
bilinear interp uses the 4-corner factored form with host-precomputed corner
products w00..w11: per sample two independent ACT->DVE chains
(ACT: a=A*w00, c=C*w10 via activation-with-scale; DVE: s1=B*w01+a,
s2=D*w11+c via scalar_tensor_tensor), then one wide 9k add s1+s2 and a
4-op max tree on DVE, all in position-major layout.  PE does q transposes
and the two 1x1 convs (q^T w0^T / x^T w1^T with an extra channel-sum
column), followed by a fused normalized-correlation epilogue (ACT
square-accumulate, DVE product-accumulate, final combine once per core).
"""

import numpy as np

B, C, H, W = 4, 256, 128, 128
PAD = 8
Hp, Wp = H + 2 * PAD, W + 2 * PAD
ROWS = 64            # rows per core (H split in 2)
N = ROWS * W         # positions per core
BLK = 128            # positions per block (= one row)
NBLK = N // BLK      # 64
GRP = 2              # blocks per gather group
NG = NBLK // GRP     # 32
NIDX = GRP * 9 * BLK  # gather indices per group (2 blk * 9 k * 128 pos)
EPS = 1e-5

# row-pair layout: A = pairs (0,1),(2,3),...,(142,143); B = (1,2),...,(141,142)
NPA = (Hp // 2) * Wp          # 72*144 = 10368 elements (each 512 fp16)
NPB = (Hp // 2 - 1) * Wp      # 71*144 = 10224
NPT = NPA + NPB               # 20592  (< int16 max)

_NC_CACHE = {}


def _build_nc(has_bias: bool, n_groups: int = NG):
    import concourse.bacc as bacc
    import concourse.bass as bass
    import concourse.tile as tile
    import concourse.mybir as mybir
    from concourse import library_config

    f16 = mybir.dt.float16
    f32 = mybir.dt.float32
    i16 = mybir.dt.int16
    Alu = mybir.AluOpType
    Act = mybir.ActivationFunctionType

    nc = bacc.Bacc("TRN2", target_bir_lowering=False, debug=False, num_devices=8)

    xt = nc.dram_tensor("xt", [(NPT + 1) * 512], f16, kind="ExternalInput")
    xk = nc.dram_tensor("xk", [2, 128, N], f16, kind="ExternalInput")
    idx = nc.dram_tensor("idx", [n_groups, 128, NIDX // 16], i16, kind="ExternalInput")
    w4 = nc.dram_tensor("w4", [n_groups, 128, GRP, 18], f16, kind="ExternalInput")
    w4f = nc.dram_tensor("w4f", [n_groups, 128, GRP, 18], f32, kind="ExternalInput")
    w0t = nc.dram_tensor("w0t", [2, 128, 257], f16, kind="ExternalInput")
    w1t = nc.dram_tensor("w1t", [2, 128, 257], f16, kind="ExternalInput")
    idmat = nc.dram_tensor("idmat", [128, 128], f16, kind="ExternalInput")
    if has_bias:
        qb = nc.dram_tensor("qb", [128, 257], f32, kind="ExternalInput")
        kb = nc.dram_tensor("kb", [128, 257], f32, kind="ExternalInput")
    nblk_t = n_groups * GRP
    o = nc.dram_tensor("o", [128, nblk_t], f32, kind="ExternalOutput")

    # overlapping-window gather view: element j = xt[j*512 : j*512+1024]
    xt_view = bass.AP(tensor=xt[:].tensor, offset=0, ap=[[512, NPT], [1, 1024]])

    with tile.TileContext(nc) as tc:
        import contextlib

        with contextlib.ExitStack() as ctx:
            consts = ctx.enter_context(tc.tile_pool(name="consts", bufs=1))
            gpool = ctx.enter_context(tc.tile_pool(name="gath", bufs=3))
            iopool = ctx.enter_context(tc.tile_pool(name="io", bufs=3))
            work = ctx.enter_context(tc.tile_pool(name="work", bufs=3))
            kpool = ctx.enter_context(tc.tile_pool(name="kp", bufs=6))
            pspool = ctx.enter_context(tc.tile_pool(name="ps", bufs=2, space="PSUM"))

            # first group's gather indices go out before anything else so the
            # first dma_gather can start as early as possible (scalar HWDGE
            # queue, parallel to the sync-queue const loads)
            idx0_t = iopool.tile([128, NIDX // 16], i16, tag="idx")
            nc.sync.dma_start(out=idx0_t, in_=idx[0])

            # constants
            w0t_sb = consts.tile([128, 2, 257], f16)
            nc.sync.dma_start(out=w0t_sb, in_=w0t[:, :, :].rearrange("t p o -> p t o"))
            w1t_sb = consts.tile([128, 2, 257], f16)
            nc.sync.dma_start(out=w1t_sb, in_=w1t[:, :, :].rearrange("t p o -> p t o"))
            ident = consts.tile([128, 128], f16)
            nc.sync.dma_start(out=ident, in_=idmat[:, :])
            if has_bias:
                qb_sb = consts.tile([128, 257], f32)
                nc.sync.dma_start(out=qb_sb, in_=qb[:, :])
                kb_sb = consts.tile([128, 257], f32)
                nc.sync.dma_start(out=kb_sb, in_=kb[:, :])

            # per-block scalar accumulators [128 pos, NBLK]
            sqs = consts.tile([128, nblk_t], f32, tag="sqs")
            sks = consts.tile([128, nblk_t], f32, tag="sks")
            sqks = consts.tile([128, nblk_t], f32, tag="sqks")
            sQs = consts.tile([128, nblk_t], f32, tag="sQs")
            sKs = consts.tile([128, nblk_t], f32, tag="sKs")

            nc.gpsimd.load_library(library_config.mlp)

            for g in range(n_groups):
                if g == 0:
                    idx_t = idx0_t
                else:
                    idx_t = iopool.tile([128, NIDX // 16], i16, tag="idx")
                    nc.sync.dma_start(out=idx_t, in_=idx[g])
                w4_t = iopool.tile([128, GRP, 18], f16, tag="w4")
                nc.sync.dma_start(out=w4_t, in_=w4[g])
                w4f_t = iopool.tile([128, GRP, 18], f32, tag="w4f")
                nc.sync.dma_start(out=w4f_t, in_=w4f[g])
                xk_t = iopool.tile([128, 2, GRP * BLK], f16, tag="xk")
                nc.sync.dma_start(
                    out=xk_t, in_=xk[:, :, g * GRP * BLK:(g + 1) * GRP * BLK]
                    .rearrange("t p n -> p t n")
                )
                # gat[p, blk, k, x, r, c]: 2x2 patch (x = column, r = row)
                gat = gpool.tile([128, GRP, 9, 2, 2, 256], f16, tag="gat")
                if g == 0:
                    # split the first gathers so compute starts sooner:
                    # blk0 in three 3-k chunks, blk1 whole
                    for k0 in range(0, 9, 3):
                        nch = 3 * BLK
                        nc.gpsimd.dma_gather(
                            gat[:, 0, k0:k0 + 3].rearrange(
                                "p k x r c -> p k (x r c)"),
                            xt_view,
                            idx_t[:, k0 * (BLK // 16):(k0 + 3) * (BLK // 16)],
                            nch, nch, 1024, elem_step=512,
                            single_packet=False,
                        )
                    nido2 = NIDX // GRP
                    nc.gpsimd.dma_gather(
                        gat[:, 1].rearrange("p k x r c -> p k (x r c)"),
                        xt_view,
                        idx_t[:, (nido2 // 16):],
                        nido2, nido2, 1024, elem_step=512,
                        single_packet=False,
                    )
                else:
                    nc.gpsimd.dma_gather(
                        gat.rearrange("p a k x r c -> p (a k) (x r c)"),
                        xt_view, idx_t, NIDX, NIDX, 1024, elem_step=512,
                        single_packet=False,
                    )

                for blk in range(GRP):
                    nblk = g * GRP + blk
                    s1_all = kpool.tile([128, 9, 256], f16, tag="s1a", bufs=2)
                    s2_all = kpool.tile([128, 9, 256], f16, tag="s2a", bufs=2)
                    for k in range(9):
                        # 2x2 patch corners, each contiguous 256:
                        # A=(x0,y0) B=(x0,y1) C=(x1,y0) D=(x1,y1)
                        A = gat[:, blk, k, 0, 0, :]
                        Bc = gat[:, blk, k, 0, 1, :]
                        Cc = gat[:, blk, k, 1, 0, :]
                        D = gat[:, blk, k, 1, 1, :]
                        # host-precomputed corner products:
                        # w4f (f32, ACT): wa=(1-fx)(1-fy), wc=fx(1-fy)
                        # w4  (f16, DVE): wb=(1-fx)fy,    wd=fx*fy
                        wa = w4f_t[:, blk, k:k + 1]
                        wc = w4f_t[:, blk, 9 + k:10 + k]
                        wb = w4_t[:, blk, k:k + 1]
                        wd = w4_t[:, blk, 9 + k:10 + k]
                        # two independent ACT->DVE chains per k
                        a_t = kpool.tile([128, 256], f16, tag="ta", bufs=8)
                        nc.scalar.activation(a_t, A, Act.Copy, bias=0.0, scale=wa)
                        c_t = kpool.tile([128, 256], f16, tag="tc", bufs=8)
                        nc.scalar.activation(c_t, Cc, Act.Copy, bias=0.0, scale=wc)
                        nc.vector.scalar_tensor_tensor(
                            s1_all[:, k, :], Bc, wb, a_t, Alu.mult, Alu.add
                        )
                        nc.vector.scalar_tensor_tensor(
                            s2_all[:, k, :], D, wd, c_t, Alu.mult, Alu.add
                        )
                    # one wide add (2304 elems), then 9-way max tree on DVE
                    nc.vector.tensor_tensor(s1_all, s1_all, s2_all, Alu.add)
                    nc.vector.tensor_tensor(
                        s1_all[:, 0:4, :], s1_all[:, 0:4, :], s1_all[:, 4:8, :],
                        Alu.max,
                    )
                    nc.vector.tensor_tensor(
                        s1_all[:, 0:2, :], s1_all[:, 0:2, :], s1_all[:, 2:4, :],
                        Alu.max,
                    )
                    nc.vector.tensor_tensor(
                        s1_all[:, 0, :], s1_all[:, 0, :], s1_all[:, 1, :], Alu.max
                    )
                    q_t = work.tile([128, 256], f16, tag="q")
                    nc.vector.tensor_tensor(
                        q_t, s1_all[:, 0, :], s1_all[:, 8, :], Alu.max
                    )

                    # transpose q -> qT (c-major) via PE
                    qt_ps = pspool.tile([128, 2, 128], f16, tag="qt")
                    for t in range(2):
                        nc.tensor.transpose(
                            qt_ps[:, t, :], q_t[:, t * 128:(t + 1) * 128], ident
                        )
                    qt_sb = work.tile([128, 2, 128], f16, tag="qt_sb")
                    nc.vector.tensor_copy(qt_sb, qt_ps)

                    # Q = qT^T @ w0t  -> [128 pos, 257] (col 256 = sum_o Q)
                    Q_ps = pspool.tile([128, 257], f32, tag="Q", bufs=3)
                    for t in range(2):
                        nc.tensor.matmul(
                            Q_ps, qt_sb[:, t, :], w0t_sb[:, t, :],
                            start=(t == 0), stop=(t == 1),
                        )
                    K_ps = pspool.tile([128, 257], f32, tag="K", bufs=3)
                    for t in range(2):
                        nc.tensor.matmul(
                            K_ps, xk_t[:, t, blk * BLK:(blk + 1) * BLK],
                            w1t_sb[:, t, :], start=(t == 0), stop=(t == 1),
                        )
                    if has_bias:
                        nc.vector.tensor_tensor(Q_ps, Q_ps, qb_sb, Alu.add)
                        nc.vector.tensor_tensor(K_ps, K_ps, kb_sb, Alu.add)

                    # epilogue reductions
                    col = slice(nblk, nblk + 1)
                    act_scr = work.tile([128, 256], f16, tag="act_scr")
                    nc.scalar.activation(
                        act_scr, Q_ps[:, 0:256], Act.Square,
                        accum_out=sqs[:, col],
                    )
                    K_sb = work.tile([128, 256], f16, tag="K_sb")
                    nc.scalar.copy(K_sb, K_ps[:, 0:256])
                    nc.scalar.activation(
                        act_scr, K_ps[:, 0:256], Act.Square, accum_out=sks[:, col],
                    )
                    dve_scr = work.tile([128, 256], f16, tag="dve_scr")
                    nc.vector.scalar_tensor_tensor(
                        dve_scr, Q_ps[:, 0:256], 0.0, K_sb, Alu.bypass, Alu.mult,
                        accum_out=sqks[:, col],
                    )
                    nc.vector.tensor_copy(sQs[:, col], Q_ps[:, 256:257])
                    nc.vector.tensor_copy(sKs[:, col], K_ps[:, 256:257])

            # final combine over [128, NBLK]
            tmp = consts.tile([128, nblk_t], f32, tag="tmp")
            num = consts.tile([128, nblk_t], f32, tag="num")
            dq = consts.tile([128, nblk_t], f32, tag="dq")
            dk = consts.tile([128, nblk_t], f32, tag="dk")
            out_t = consts.tile([128, nblk_t], f32, tag="out")
            inv_c = -1.0 / C
            # num = sqk - sQ*sK/C
            nc.vector.tensor_tensor(tmp, sQs, sKs, Alu.mult)
            nc.vector.scalar_tensor_tensor(num, tmp, inv_c, sqks, Alu.mult, Alu.add)
            # dq = sq - sQ^2/C + eps
            nc.vector.tensor_tensor(tmp, sQs, sQs, Alu.mult)
            nc.vector.scalar_tensor_tensor(dq, tmp, inv_c, sqs, Alu.mult, Alu.add)
            nc.vector.tensor_scalar(dq, dq, EPS, None, Alu.add)
            nc.vector.tensor_tensor(tmp, sKs, sKs, Alu.mult)
            nc.vector.scalar_tensor_tensor(dk, tmp, inv_c, sks, Alu.mult, Alu.add)
            nc.vector.tensor_scalar(dk, dk, EPS, None, Alu.add)
            # out = num / sqrt(dq*dk)
            nc.vector.tensor_tensor(tmp, dq, dk, Alu.mult)
            nc.scalar.activation(tmp, tmp, Act.Sqrt)
            nc.vector.reciprocal(tmp, tmp)
            nc.vector.tensor_tensor(out_t, num, tmp, Alu.mult)
            nc.sync.dma_start(out=o[:, :], in_=out_t)

    nc.compile()
    return nc


def _get_nc(has_bias: bool):
    if has_bias not in _NC_CACHE:
        _NC_CACHE[has_bias] = _build_nc(has_bias)
    return _NC_CACHE[has_bias]


def _build_pair(x_b):
    """Row-pair channel-last layout: A-pairs (even start) then B-pairs (odd),
    with one trailing 512-elem pad element."""
    xp = np.zeros((Hp, Wp, C), np.float16)
    xp[PAD:PAD + H, PAD:PAD + W, :] = x_b.transpose(1, 2, 0)
    pa = xp.reshape(Hp // 2, 2, Wp, C).transpose(0, 2, 1, 3)  # [p, x, r, c]
    pb = xp[1:Hp - 1].reshape(Hp // 2 - 1, 2, Wp, C).transpose(0, 2, 1, 3)
    flat = np.empty(((NPT + 1) * 512,), np.float16)
    flat[:NPA * 512] = pa.reshape(-1)
    flat[NPA * 512:NPT * 512] = pb.reshape(-1)
    flat[NPT * 512:] = 0
    return flat


def _prep_core(x_b, off_b, h0):
    """Host-side shard prep for one core: indices, weights, fp16 layouts."""
    ys, xs = np.meshgrid(
        np.arange(h0, h0 + ROWS), np.arange(W), indexing="ij"
    )
    ys = ys.reshape(-1).astype(np.float32)
    xs = xs.reshape(-1).astype(np.float32)

    idx_all = np.empty((N, 9), np.int32)
    fy_all = np.empty((N, 9), np.float32)
    fx_all = np.empty((N, 9), np.float32)
    for k in range(9):
        kh, kw = k // 3 - 1, k % 3 - 1
        iy = ys.astype(np.int32)
        ix = xs.astype(np.int32)
        py = ys + kh + off_b[2 * k, iy, ix]
        px = xs + kw + off_b[2 * k + 1, iy, ix]
        y0 = np.clip(np.floor(py).astype(np.int32), -PAD, H + PAD - 2)
        x0 = np.clip(np.floor(px).astype(np.int32), -PAD, W + PAD - 2)
        fy_all[:, k] = py - y0
        fx_all[:, k] = px - x0
        yp = y0 + PAD  # [0, 142]
        xpp = x0 + PAD
        even = (yp % 2) == 0
        idx_all[:, k] = np.where(
            even, (yp // 2) * Wp + xpp, NPA + (yp // 2) * Wp + xpp
        )

    # idx tensor [NG, 128, NIDX//16]: slot m = j*128 + p, j = blk*9 + k
    idx_np = np.empty((NG, 128, NIDX // 16), np.int16)
    for g in range(NG):
        slots = np.empty((GRP * 9, BLK), np.int32)
        for blk in range(GRP):
            base = (g * GRP + blk) * BLK
            for k in range(9):
                slots[blk * 9 + k, :] = idx_all[base:base + BLK, k]
        wrapped = slots.reshape(-1).reshape(NIDX // 16, 16).T  # [16, cols]
        idx_np[g] = np.tile(wrapped, (8, 1)).astype(np.int16)

    # corner-product weights: fp16 [NG, 128, GRP, 18] (wb, wd) for DVE,
    # fp32 [.., 18] (wa, wc) for ACT scales
    w4_np = np.empty((NG, 128, GRP, 18), np.float16)
    w4f_np = np.empty((NG, 128, GRP, 18), np.float32)
    fy = fy_all.reshape(NBLK, BLK, 9)
    fx = fx_all.reshape(NBLK, BLK, 9)
    for g in range(NG):
        for blk in range(GRP):
            nb = g * GRP + blk
            w4_np[g, :, blk, 0:9] = (1.0 - fx[nb]) * fy[nb]
            w4_np[g, :, blk, 9:18] = fx[nb] * fy[nb]
            w4f_np[g, :, blk, 0:9] = (1.0 - fx[nb]) * (1.0 - fy[nb])
            w4f_np[g, :, blk, 9:18] = fx[nb] * (1.0 - fy[nb])

    xk_np = np.ascontiguousarray(
        x_b.reshape(2, 128, H, W)[:, :, h0:h0 + ROWS, :].reshape(2, 128, N)
    ).astype(np.float16)
    return idx_np, w4_np, w4f_np, xk_np


def _build_in_maps(x, offset, w0, b0, w1, b1, has_bias):
    w0t_np = np.concatenate([w0.T, w0.sum(0)[:, None]], 1).astype(np.float16)
    w1t_np = np.concatenate([w1.T, w1.sum(0)[:, None]], 1).astype(np.float16)
    w0t_np = np.ascontiguousarray(w0t_np.reshape(2, 128, 257))
    w1t_np = np.ascontiguousarray(w1t_np.reshape(2, 128, 257))

    in_maps = []
    xt_cache = {}
    for core in range(8):
        b, half = core // 2, core % 2
        h0 = ROWS * half
        if b not in xt_cache:
            xt_cache[b] = _build_pair(x[b])
        idx_np, w4_np, w4f_np, xk_np = _prep_core(x[b], offset[b], h0)
        m = {
            "idmat": np.eye(128, dtype=np.float16),
            "xt": xt_cache[b],
            "xk": xk_np,
            "idx": idx_np,
            "w4": w4_np,
            "w4f": w4f_np,
            "w0t": w0t_np,
            "w1t": w1t_np,
        }
        if has_bias:
            qb_np = np.concatenate([b0, [b0.sum()]]).astype(np.float32)
            kb_np = np.concatenate([b1, [b1.sum()]]).astype(np.float32)
            m["qb"] = np.tile(qb_np[None, :], (128, 1))
            m["kb"] = np.tile(kb_np[None, :], (128, 1))
        in_maps.append(m)
    return in_maps


def kernel(x, offset, w0, b0, w1, b1):
    from concourse.bass_utils import run_bass_kernel_spmd

    x = np.asarray(x, np.float32)
    offset = np.asarray(offset, np.float32)
    w0 = np.asarray(w0, np.float32)
    w1 = np.asarray(w1, np.float32)
    b0 = np.asarray(b0, np.float32)
    b1 = np.asarray(b1, np.float32)

    has_bias = bool(np.any(b0)) or bool(np.any(b1))
    nc = _get_nc(has_bias)
    in_maps = _build_in_maps(x, offset, w0, b0, w1, b1, has_bias)

    res = run_bass_kernel_spmd(nc, in_maps, core_ids=list(range(8)))

    out = np.empty((B, 1, H, W), np.float32)
    for core in range(8):
        b, half = core // 2, core % 2
        h0 = ROWS * half
        o = res.results[core]["o"]  # [128 pos(x), 64 rows]
        out[b, 0, h0:h0 + ROWS, :] = o.T
    return out


# revision 50
# speedup vs baseline: 1.0240x; 1.0100x over previous
"""Trainium2 Bass kernel for nn_DeformSpaceAttentionv5 (deformable 3x3 unfold
+ per-channel max + two 1x1 convs + channel-norm dot product).

Contract: kernel(**inputs) takes the FULL inputs (x [4,256,128,128] f32,
offset [4,18,128,128] f32, w0/w1 [256,256] f32, b0/b1 [256] f32) and returns
the FULL output [4,1,128,128] f32.

Strategy (pure data parallel over 8 NeuronCores): core = (batch, H-half).
The padded image is stored in DRAM twice in a "row-pair" channel-last layout
(pairs starting at even rows and at odd rows), so that ONE gather descriptor
fetches the full 2x2 bilinear patch (2 rows x 2 cols x 256 ch = 1024 fp16
contiguous).  SWDGE dma_gather brings 9 such patches per position.  The
bilinear interp uses the 4-corner factored form with host-precomputed corner
products w00..w11: per sample two independent ACT->DVE chains
(ACT: a=A*w00, c=C*w10 via activation-with-scale; DVE: s1=B*w01+a,
s2=D*w11+c via scalar_tensor_tensor), then one wide 9k add s1+s2 and a
4-op max tree on DVE, all in position-major layout.  PE does q transposes
and the two 1x1 convs (q^T w0^T / x^T w1^T with an extra channel-sum
column), followed by a fused normalized-correlation epilogue (ACT
square-accumulate, DVE product-accumulate, final combine once per core).
"""

import numpy as np

B, C, H, W = 4, 256, 128, 128
PAD = 8
Hp, Wp = H + 2 * PAD, W + 2 * PAD
ROWS = 64            # rows per core (H split in 2)
N = ROWS * W         # positions per core
BLK = 128            # positions per block (= one row)
NBLK = N // BLK      # 64
GRP = 2              # blocks per gather group
NG = NBLK // GRP     # 32
NIDX = GRP * 9 * BLK  # gather indices per group (2 blk * 9 k * 128 pos)
EPS = 1e-5

# row-pair layout: A = pairs (0,1),(2,3),...,(142,143); B = (1,2),...,(141,142)
NPA = (Hp // 2) * Wp          # 72*144 = 10368 elements (each 512 fp16)
NPB = (Hp // 2 - 1) * Wp      # 71*144 = 10224
NPT = NPA + NPB               # 20592  (< int16 max)

_NC_CACHE = {}


def _build_nc(has_bias: bool, n_groups: int = NG):
    import concourse.bacc as bacc
    import concourse.bass as bass
    import concourse.tile as tile
    import concourse.mybir as mybir
    from concourse import library_config

    f16 = mybir.dt.float16
    f32 = mybir.dt.float32
    i16 = mybir.dt.int16
    Alu = mybir.AluOpType
    Act = mybir.ActivationFunctionType

    nc = bacc.Bacc("TRN2", target_bir_lowering=False, debug=False, num_devices=8)

    xt = nc.dram_tensor("xt", [(NPT + 1) * 512], f16, kind="ExternalInput")
    xk = nc.dram_tensor("xk", [2, 128, N], f16, kind="ExternalInput")
    idx = nc.dram_tensor("idx", [n_groups, 128, NIDX // 16], i16, kind="ExternalInput")
    w4 = nc.dram_tensor("w4", [n_groups, 128, GRP, 18], f16, kind="ExternalInput")
    w4f = nc.dram_tensor("w4f", [n_groups, 128, GRP, 18], f32, kind="ExternalInput")
    w0t = nc.dram_tensor("w0t", [2, 128, 257], f16, kind="ExternalInput")
    w1t = nc.dram_tensor("w1t", [2, 128, 257], f16, kind="ExternalInput")
    idmat = nc.dram_tensor("idmat", [128, 128], f16, kind="ExternalInput")
    if has_bias:
        qb = nc.dram_tensor("qb", [128, 257], f32, kind="ExternalInput")
        kb = nc.dram_tensor("kb", [128, 257], f32, kind="ExternalInput")
    nblk_t = n_groups * GRP
    o = nc.dram_tensor("o", [128, nblk_t], f32, kind="ExternalOutput")

    # overlapping-window gather view: element j = xt[j*512 : j*512+1024]
    xt_view = bass.AP(tensor=xt[:].tensor, offset=0, ap=[[512, NPT], [1, 1024]])

    with tile.TileContext(nc) as tc:
        import contextlib

        with contextlib.ExitStack() as ctx:
            consts = ctx.enter_context(tc.tile_pool(name="consts", bufs=1))
            gpool = ctx.enter_context(tc.tile_pool(name="gath", bufs=3))
            iopool = ctx.enter_context(tc.tile_pool(name="io", bufs=3))
            work = ctx.enter_context(tc.tile_pool(name="work", bufs=3))
            kpool = ctx.enter_context(tc.tile_pool(name="kp", bufs=6))
            pspool = ctx.enter_context(tc.tile_pool(name="ps", bufs=2, space="PSUM"))

            # first group's gather indices go out before anything else so the
            # first dma_gather can start as early as possible (scalar HWDGE
            # queue, parallel to the sync-queue const loads)
            idx0_t = iopool.tile([128, NIDX // 16], i16, tag="idx")
            nc.sync.dma_start(out=idx0_t, in_=idx[0])

            # constants
            w0t_sb = consts.tile([128, 2, 257], f16)
            nc.sync.dma_start(out=w0t_sb, in_=w0t[:, :, :].rearrange("t p o -> p t o"))
            w1t_sb = consts.tile([128, 2, 257], f16)
            nc.sync.dma_start(out=w1t_sb, in_=w1t[:, :, :].rearrange("t p o -> p t o"))
            ident = consts.tile([128, 128], f16)
            nc.sync.dma_start(out=ident, in_=idmat[:, :])
            if has_bias:
                qb_sb = consts.tile([128, 257], f32)
                nc.sync.dma_start(out=qb_sb, in_=qb[:, :])
                kb_sb = consts.tile([128, 257], f32)
                nc.sync.dma_start(out=kb_sb, in_=kb[:, :])

            # per-block scalar accumulators [128 pos, NBLK]
            sqs = consts.tile([128, nblk_t], f32, tag="sqs")
            sks = consts.tile([128, nblk_t], f32, tag="sks")
            sqks = consts.tile([128, nblk_t], f32, tag="sqks")
            sQs = consts.tile([128, nblk_t], f32, tag="sQs")
            sKs = consts.tile([128, nblk_t], f32, tag="sKs")

            nc.gpsimd.load_library(library_config.mlp)

            for g in range(n_groups):
                if g == 0:
                    idx_t = idx0_t
                else:
                    idx_t = iopool.tile([128, NIDX // 16], i16, tag="idx")
                    nc.sync.dma_start(out=idx_t, in_=idx[g])
                w4_t = iopool.tile([128, GRP, 18], f16, tag="w4")
                nc.sync.dma_start(out=w4_t, in_=w4[g])
                w4f_t = iopool.tile([128, GRP, 18], f32, tag="w4f")
                nc.sync.dma_start(out=w4f_t, in_=w4f[g])
                xk_t = iopool.tile([128, 2, GRP * BLK], f16, tag="xk")
                nc.sync.dma_start(
                    out=xk_t, in_=xk[:, :, g * GRP * BLK:(g + 1) * GRP * BLK]
                    .rearrange("t p n -> p t n")
                )
                # gat[p, blk, k, x, r, c]: 2x2 patch (x = column, r = row)
                gat = gpool.tile([128, GRP, 9, 2, 2, 256], f16, tag="gat")
                if g == 0:
                    # split the first gathers so compute starts sooner:
                    # blk0 in three 3-k chunks, blk1 whole
                    for k0 in range(0, 9, 3):
                        nch = 3 * BLK
                        nc.gpsimd.dma_gather(
                            gat[:, 0, k0:k0 + 3].rearrange(
                                "p k x r c -> p k (x r c)"),
                            xt_view,
                            idx_t[:, k0 * (BLK // 16):(k0 + 3) * (BLK // 16)],
                            nch, nch, 1024, elem_step=512,
                            single_packet=False,
                        )
                    nido2 = NIDX // GRP
                    nc.gpsimd.dma_gather(
                        gat[:, 1].rearrange("p k x r c -> p k (x r c)"),
                        xt_view,
                        idx_t[:, (nido2 // 16):],
                        nido2, nido2, 1024, elem_step=512,
                        single_packet=False,
                    )
                elif g <= 2:
                    # per-block gathers during pipeline fill: earlier data
                    # arrival for the first consumed blocks
                    nido2 = NIDX // GRP
                    for blk in range(GRP):
                        nc.gpsimd.dma_gather(
                            gat[:, blk].rearrange("p k x r c -> p k (x r c)"),
                            xt_view,
                            idx_t[:, blk * (nido2 // 16):(blk + 1) * (nido2 // 16)],
                            nido2, nido2, 1024, elem_step=512,
                            single_packet=False,
                        )
                else:
                    nc.gpsimd.dma_gather(
                        gat.rearrange("p a k x r c -> p (a k) (x r c)"),
                        xt_view, idx_t, NIDX, NIDX, 1024, elem_step=512,
                        single_packet=False,
                    )

                for blk in range(GRP):
                    nblk = g * GRP + blk
                    s1_all = kpool.tile([128, 9, 256], f16, tag="s1a", bufs=2)
                    s2_all = kpool.tile([128, 9, 256], f16, tag="s2a", bufs=2)
                    for k in range(9):
                        # 2x2 patch corners, each contiguous 256:
                        # A=(x0,y0) B=(x0,y1) C=(x1,y0) D=(x1,y1)
                        A = gat[:, blk, k, 0, 0, :]
                        Bc = gat[:, blk, k, 0, 1, :]
                        Cc = gat[:, blk, k, 1, 0, :]
                        D = gat[:, blk, k, 1, 1, :]
                        # host-precomputed corner products:
                        # w4f (f32, ACT): wa=(1-fx)(1-fy), wc=fx(1-fy)
                        # w4  (f16, DVE): wb=(1-fx)fy,    wd=fx*fy
                        wa = w4f_t[:, blk, k:k + 1]
                        wc = w4f_t[:, blk, 9 + k:10 + k]
                        wb = w4_t[:, blk, k:k + 1]
                        wd = w4_t[:, blk, 9 + k:10 + k]
                        # two independent ACT->DVE chains per k
                        a_t = kpool.tile([128, 256], f16, tag="ta", bufs=8)
                        nc.scalar.activation(a_t, A, Act.Copy, bias=0.0, scale=wa)
                        c_t = kpool.tile([128, 256], f16, tag="tc", bufs=8)
                        nc.scalar.activation(c_t, Cc, Act.Copy, bias=0.0, scale=wc)
                        nc.vector.scalar_tensor_tensor(
                            s1_all[:, k, :], Bc, wb, a_t, Alu.mult, Alu.add
                        )
                        nc.vector.scalar_tensor_tensor(
                            s2_all[:, k, :], D, wd, c_t, Alu.mult, Alu.add
                        )
                    # one wide add (2304 elems), then 9-way max tree on DVE
                    nc.vector.tensor_tensor(s1_all, s1_all, s2_all, Alu.add)
                    nc.vector.tensor_tensor(
                        s1_all[:, 0:4, :], s1_all[:, 0:4, :], s1_all[:, 4:8, :],
                        Alu.max,
                    )
                    nc.vector.tensor_tensor(
                        s1_all[:, 0:2, :], s1_all[:, 0:2, :], s1_all[:, 2:4, :],
                        Alu.max,
                    )
                    nc.vector.tensor_tensor(
                        s1_all[:, 0, :], s1_all[:, 0, :], s1_all[:, 1, :], Alu.max
                    )
                    q_t = work.tile([128, 256], f16, tag="q")
                    nc.vector.tensor_tensor(
                        q_t, s1_all[:, 0, :], s1_all[:, 8, :], Alu.max
                    )

                    # transpose q -> qT (c-major) via PE
                    qt_ps = pspool.tile([128, 2, 128], f16, tag="qt")
                    for t in range(2):
                        nc.tensor.transpose(
                            qt_ps[:, t, :], q_t[:, t * 128:(t + 1) * 128], ident
                        )
                    qt_sb = work.tile([128, 2, 128], f16, tag="qt_sb")
                    if nblk % 2 == 0:
                        nc.vector.tensor_copy(qt_sb, qt_ps)
                    else:
                        nc.scalar.copy(qt_sb, qt_ps)

                    # Q = qT^T @ w0t  -> [128 pos, 257] (col 256 = sum_o Q)
                    Q_ps = pspool.tile([128, 257], f32, tag="Q", bufs=3)
                    for t in range(2):
                        nc.tensor.matmul(
                            Q_ps, qt_sb[:, t, :], w0t_sb[:, t, :],
                            start=(t == 0), stop=(t == 1),
                        )
                    K_ps = pspool.tile([128, 257], f32, tag="K", bufs=3)
                    for t in range(2):
                        nc.tensor.matmul(
                            K_ps, xk_t[:, t, blk * BLK:(blk + 1) * BLK],
                            w1t_sb[:, t, :], start=(t == 0), stop=(t == 1),
                        )
                    if has_bias:
                        nc.vector.tensor_tensor(Q_ps, Q_ps, qb_sb, Alu.add)
                        nc.vector.tensor_tensor(K_ps, K_ps, kb_sb, Alu.add)

                    # epilogue reductions
                    col = slice(nblk, nblk + 1)
                    act_scr = work.tile([128, 256], f16, tag="act_scr")
                    nc.scalar.activation(
                        act_scr, Q_ps[:, 0:256], Act.Square,
                        accum_out=sqs[:, col],
                    )
                    K_sb = work.tile([128, 256], f16, tag="K_sb")
                    nc.scalar.copy(K_sb, K_ps[:, 0:256])
                    nc.scalar.activation(
                        act_scr, K_ps[:, 0:256], Act.Square, accum_out=sks[:, col],
                    )
                    dve_scr = work.tile([128, 256], f16, tag="dve_scr")
                    nc.vector.scalar_tensor_tensor(
                        dve_scr, Q_ps[:, 0:256], 0.0, K_sb, Alu.bypass, Alu.mult,
                        accum_out=sqks[:, col],
                    )
                    nc.vector.tensor_copy(sQs[:, col], Q_ps[:, 256:257])
                    nc.vector.tensor_copy(sKs[:, col], K_ps[:, 256:257])

            # final combine over [128, NBLK]
            tmp = consts.tile([128, nblk_t], f32, tag="tmp")
            num = consts.tile([128, nblk_t], f32, tag="num")
            dq = consts.tile([128, nblk_t], f32, tag="dq")
            dk = consts.tile([128, nblk_t], f32, tag="dk")
            out_t = consts.tile([128, nblk_t], f32, tag="out")
            inv_c = -1.0 / C
            # num = sqk - sQ*sK/C
            nc.vector.tensor_tensor(tmp, sQs, sKs, Alu.mult)
            nc.vector.scalar_tensor_tensor(num, tmp, inv_c, sqks, Alu.mult, Alu.add)
            # dq = sq - sQ^2/C + eps
            nc.vector.tensor_tensor(tmp, sQs, sQs, Alu.mult)
            nc.vector.scalar_tensor_tensor(dq, tmp, inv_c, sqs, Alu.mult, Alu.add)
            nc.vector.tensor_scalar(dq, dq, EPS, None, Alu.add)
            nc.vector.tensor_tensor(tmp, sKs, sKs, Alu.mult)
            nc.vector.scalar_tensor_tensor(dk, tmp, inv_c, sks, Alu.mult, Alu.add)
            nc.vector.tensor_scalar(dk, dk, EPS, None, Alu.add)
            # out = num / sqrt(dq*dk)
            nc.vector.tensor_tensor(tmp, dq, dk, Alu.mult)
            nc.scalar.activation(tmp, tmp, Act.Sqrt)
            nc.vector.reciprocal(tmp, tmp)
            nc.vector.tensor_tensor(out_t, num, tmp, Alu.mult)
            nc.sync.dma_start(out=o[:, :], in_=out_t)

    nc.compile()
    return nc


def _get_nc(has_bias: bool):
    if has_bias not in _NC_CACHE:
        _NC_CACHE[has_bias] = _build_nc(has_bias)
    return _NC_CACHE[has_bias]


def _build_pair(x_b):
    """Row-pair channel-last layout: A-pairs (even start) then B-pairs (odd),
    with one trailing 512-elem pad element."""
    xp = np.zeros((Hp, Wp, C), np.float16)
    xp[PAD:PAD + H, PAD:PAD + W, :] = x_b.transpose(1, 2, 0)
    pa = xp.reshape(Hp // 2, 2, Wp, C).transpose(0, 2, 1, 3)  # [p, x, r, c]
    pb = xp[1:Hp - 1].reshape(Hp // 2 - 1, 2, Wp, C).transpose(0, 2, 1, 3)
    flat = np.empty(((NPT + 1) * 512,), np.float16)
    flat[:NPA * 512] = pa.reshape(-1)
    flat[NPA * 512:NPT * 512] = pb.reshape(-1)
    flat[NPT * 512:] = 0
    return flat


def _prep_core(x_b, off_b, h0):
    """Host-side shard prep for one core: indices, weights, fp16 layouts."""
    ys, xs = np.meshgrid(
        np.arange(h0, h0 + ROWS), np.arange(W), indexing="ij"
    )
    ys = ys.reshape(-1).astype(np.float32)
    xs = xs.reshape(-1).astype(np.float32)

    idx_all = np.empty((N, 9), np.int32)
    fy_all = np.empty((N, 9), np.float32)
    fx_all = np.empty((N, 9), np.float32)
    for k in range(9):
        kh, kw = k // 3 - 1, k % 3 - 1
        iy = ys.astype(np.int32)
        ix = xs.astype(np.int32)
        py = ys + kh + off_b[2 * k, iy, ix]
        px = xs + kw + off_b[2 * k + 1, iy, ix]
        y0 = np.clip(np.floor(py).astype(np.int32), -PAD, H + PAD - 2)
        x0 = np.clip(np.floor(px).astype(np.int32), -PAD, W + PAD - 2)
        fy_all[:, k] = py - y0
        fx_all[:, k] = px - x0
        yp = y0 + PAD  # [0, 142]
        xpp = x0 + PAD
        even = (yp % 2) == 0
        idx_all[:, k] = np.where(
            even, (yp // 2) * Wp + xpp, NPA + (yp // 2) * Wp + xpp
        )

    # idx tensor [NG, 128, NIDX//16]: slot m = j*128 + p, j = blk*9 + k
    idx_np = np.empty((NG, 128, NIDX // 16), np.int16)
    for g in range(NG):
        slots = np.empty((GRP * 9, BLK), np.int32)
        for blk in range(GRP):
            base = (g * GRP + blk) * BLK
            for k in range(9):
                slots[blk * 9 + k, :] = idx_all[base:base + BLK, k]
        wrapped = slots.reshape(-1).reshape(NIDX // 16, 16).T  # [16, cols]
        idx_np[g] = np.tile(wrapped, (8, 1)).astype(np.int16)

    # corner-product weights: fp16 [NG, 128, GRP, 18] (wb, wd) for DVE,
    # fp32 [.., 18] (wa, wc) for ACT scales
    w4_np = np.empty((NG, 128, GRP, 18), np.float16)
    w4f_np = np.empty((NG, 128, GRP, 18), np.float32)
    fy = fy_all.reshape(NBLK, BLK, 9)
    fx = fx_all.reshape(NBLK, BLK, 9)
    for g in range(NG):
        for blk in range(GRP):
            nb = g * GRP + blk
            w4_np[g, :, blk, 0:9] = (1.0 - fx[nb]) * fy[nb]
            w4_np[g, :, blk, 9:18] = fx[nb] * fy[nb]
            w4f_np[g, :, blk, 0:9] = (1.0 - fx[nb]) * (1.0 - fy[nb])
            w4f_np[g, :, blk, 9:18] = fx[nb] * (1.0 - fy[nb])

    xk_np = np.ascontiguousarray(
        x_b.reshape(2, 128, H, W)[:, :, h0:h0 + ROWS, :].reshape(2, 128, N)
    ).astype(np.float16)
    return idx_np, w4_np, w4f_np, xk_np


def _build_in_maps(x, offset, w0, b0, w1, b1, has_bias):
    w0t_np = np.concatenate([w0.T, w0.sum(0)[:, None]], 1).astype(np.float16)
    w1t_np = np.concatenate([w1.T, w1.sum(0)[:, None]], 1).astype(np.float16)
    w0t_np = np.ascontiguousarray(w0t_np.reshape(2, 128, 257))
    w1t_np = np.ascontiguousarray(w1t_np.reshape(2, 128, 257))

    in_maps = []
    xt_cache = {}
    for core in range(8):
        b, half = core // 2, core % 2
        h0 = ROWS * half
        if b not in xt_cache:
            xt_cache[b] = _build_pair(x[b])
        idx_np, w4_np, w4f_np, xk_np = _prep_core(x[b], offset[b], h0)
        m = {
            "idmat": np.eye(128, dtype=np.float16),
            "xt": xt_cache[b],
            "xk": xk_np,
            "idx": idx_np,
            "w4": w4_np,
            "w4f": w4f_np,
            "w0t": w0t_np,
            "w1t": w1t_np,
        }
        if has_bias:
            qb_np = np.concatenate([b0, [b0.sum()]]).astype(np.float32)
            kb_np = np.concatenate([b1, [b1.sum()]]).astype(np.float32)
            m["qb"] = np.tile(qb_np[None, :], (128, 1))
            m["kb"] = np.tile(kb_np[None, :], (128, 1))
        in_maps.append(m)
    return in_maps


def kernel(x, offset, w0, b0, w1, b1):
    from concourse.bass_utils import run_bass_kernel_spmd

    x = np.asarray(x, np.float32)
    offset = np.asarray(offset, np.float32)
    w0 = np.asarray(w0, np.float32)
    w1 = np.asarray(w1, np.float32)
    b0 = np.asarray(b0, np.float32)
    b1 = np.asarray(b1, np.float32)

    has_bias = bool(np.any(b0)) or bool(np.any(b1))
    nc = _get_nc(has_bias)
    in_maps = _build_in_maps(x, offset, w0, b0, w1, b1, has_bias)

    res = run_bass_kernel_spmd(nc, in_maps, core_ids=list(range(8)))

    out = np.empty((B, 1, H, W), np.float32)
    for core in range(8):
        b, half = core // 2, core % 2
        h0 = ROWS * half
        o = res.results[core]["o"]  # [128 pos(x), 64 rows]
        out[b, 0, h0:h0 + ROWS, :] = o.T
    return out


# revision 51
# speedup vs baseline: 1.0296x; 1.0055x over previous
"""Trainium2 Bass kernel for nn_DeformSpaceAttentionv5 (deformable 3x3 unfold
+ per-channel max + two 1x1 convs + channel-norm dot product).

Contract: kernel(**inputs) takes the FULL inputs (x [4,256,128,128] f32,
offset [4,18,128,128] f32, w0/w1 [256,256] f32, b0/b1 [256] f32) and returns
the FULL output [4,1,128,128] f32.

Strategy (pure data parallel over 8 NeuronCores): core = (batch, H-half).
The padded image is stored in DRAM twice in a "row-pair" channel-last layout
(pairs starting at even rows and at odd rows), so that ONE gather descriptor
fetches the full 2x2 bilinear patch (2 rows x 2 cols x 256 ch = 1024 fp16
contiguous).  SWDGE dma_gather brings 9 such patches per position.  The
bilinear interp uses the 4-corner factored form with host-precomputed corner
products w00..w11: per sample two independent ACT->DVE chains
(ACT: a=A*w00, c=C*w10 via activation-with-scale; DVE: s1=B*w01+a,
s2=D*w11+c via scalar_tensor_tensor), then one wide 9k add s1+s2 and a
4-op max tree on DVE, all in position-major layout.  PE does q transposes
and the two 1x1 convs (q^T w0^T / x^T w1^T with an extra channel-sum
column), followed by a fused normalized-correlation epilogue (ACT
square-accumulate, DVE product-accumulate, final combine once per core).
"""

import numpy as np

B, C, H, W = 4, 256, 128, 128
PAD = 8
Hp, Wp = H + 2 * PAD, W + 2 * PAD
ROWS = 64            # rows per core (H split in 2)
N = ROWS * W         # positions per core
BLK = 128            # positions per block (= one row)
NBLK = N // BLK      # 64
GRP = 2              # blocks per gather group
NG = NBLK // GRP     # 32
NIDX = GRP * 9 * BLK  # gather indices per group (2 blk * 9 k * 128 pos)
EPS = 1e-5

# row-pair layout: A = pairs (0,1),(2,3),...,(142,143); B = (1,2),...,(141,142)
NPA = (Hp // 2) * Wp          # 72*144 = 10368 elements (each 512 fp16)
NPB = (Hp // 2 - 1) * Wp      # 71*144 = 10224
NPT = NPA + NPB               # 20592  (< int16 max)

_NC_CACHE = {}


def _build_nc(has_bias: bool, n_groups: int = NG):
    import concourse.bacc as bacc
    import concourse.bass as bass
    import concourse.tile as tile
    import concourse.mybir as mybir
    from concourse import library_config

    f16 = mybir.dt.float16
    f32 = mybir.dt.float32
    i16 = mybir.dt.int16
    Alu = mybir.AluOpType
    Act = mybir.ActivationFunctionType

    nc = bacc.Bacc("TRN2", target_bir_lowering=False, debug=False, num_devices=8)

    xt = nc.dram_tensor("xt", [(NPT + 1) * 512], f16, kind="ExternalInput")
    xk = nc.dram_tensor("xk", [2, 128, N], f16, kind="ExternalInput")
    idx = nc.dram_tensor("idx", [n_groups, 128, NIDX // 16], i16, kind="ExternalInput")
    w4 = nc.dram_tensor("w4", [n_groups, 128, GRP, 18], f16, kind="ExternalInput")
    w4f = nc.dram_tensor("w4f", [n_groups, 128, GRP, 18], f32, kind="ExternalInput")
    w0t = nc.dram_tensor("w0t", [2, 128, 257], f16, kind="ExternalInput")
    w1t = nc.dram_tensor("w1t", [2, 128, 257], f16, kind="ExternalInput")
    idmat = nc.dram_tensor("idmat", [128, 128], f16, kind="ExternalInput")
    if has_bias:
        qb = nc.dram_tensor("qb", [128, 257], f32, kind="ExternalInput")
        kb = nc.dram_tensor("kb", [128, 257], f32, kind="ExternalInput")
    nblk_t = n_groups * GRP
    o = nc.dram_tensor("o", [128, nblk_t], f32, kind="ExternalOutput")

    # overlapping-window gather view: element j = xt[j*512 : j*512+1024]
    xt_view = bass.AP(tensor=xt[:].tensor, offset=0, ap=[[512, NPT], [1, 1024]])

    with tile.TileContext(nc) as tc:
        import contextlib

        with contextlib.ExitStack() as ctx:
            consts = ctx.enter_context(tc.tile_pool(name="consts", bufs=1))
            gpool = ctx.enter_context(tc.tile_pool(name="gath", bufs=3))
            iopool = ctx.enter_context(tc.tile_pool(name="io", bufs=3))
            work = ctx.enter_context(tc.tile_pool(name="work", bufs=3))
            kpool = ctx.enter_context(tc.tile_pool(name="kp", bufs=6))
            pspool = ctx.enter_context(tc.tile_pool(name="ps", bufs=2, space="PSUM"))

            # first group's gather indices go out before anything else so the
            # first dma_gather can start as early as possible (scalar HWDGE
            # queue, parallel to the sync-queue const loads)
            idx0_t = iopool.tile([128, NIDX // 16], i16, tag="idx")
            nc.sync.dma_start(out=idx0_t, in_=idx[0])

            # constants
            w0t_sb = consts.tile([128, 2, 257], f16)
            nc.sync.dma_start(out=w0t_sb, in_=w0t[:, :, :].rearrange("t p o -> p t o"))
            w1t_sb = consts.tile([128, 2, 257], f16)
            nc.sync.dma_start(out=w1t_sb, in_=w1t[:, :, :].rearrange("t p o -> p t o"))
            ident = consts.tile([128, 128], f16)
            nc.sync.dma_start(out=ident, in_=idmat[:, :])
            if has_bias:
                qb_sb = consts.tile([128, 257], f32)
                nc.sync.dma_start(out=qb_sb, in_=qb[:, :])
                kb_sb = consts.tile([128, 257], f32)
                nc.sync.dma_start(out=kb_sb, in_=kb[:, :])

            # per-block scalar accumulators [128 pos, NBLK]
            sqs = consts.tile([128, nblk_t], f32, tag="sqs")
            sks = consts.tile([128, nblk_t], f32, tag="sks")
            sqks = consts.tile([128, nblk_t], f32, tag="sqks")
            sQs = consts.tile([128, nblk_t], f32, tag="sQs")
            sKs = consts.tile([128, nblk_t], f32, tag="sKs")

            nc.gpsimd.load_library(library_config.mlp)

            for g in range(n_groups):
                if g == 0:
                    idx_t = idx0_t
                else:
                    idx_t = iopool.tile([128, NIDX // 16], i16, tag="idx")
                    nc.sync.dma_start(out=idx_t, in_=idx[g])
                w4_t = iopool.tile([128, GRP, 18], f16, tag="w4")
                nc.sync.dma_start(out=w4_t, in_=w4[g])
                w4f_t = iopool.tile([128, GRP, 18], f32, tag="w4f")
                nc.sync.dma_start(out=w4f_t, in_=w4f[g])
                xk_t = iopool.tile([128, 2, GRP * BLK], f16, tag="xk")
                nc.sync.dma_start(
                    out=xk_t, in_=xk[:, :, g * GRP * BLK:(g + 1) * GRP * BLK]
                    .rearrange("t p n -> p t n")
                )
                # gat[p, blk, k, x, r, c]: 2x2 patch (x = column, r = row)
                gat = gpool.tile([128, GRP, 9, 2, 2, 256], f16, tag="gat")
                if g == 0:
                    # split the first gathers so compute starts sooner:
                    # blk0 in three 3-k chunks, blk1 whole
                    for k0 in range(0, 9, 3):
                        nch = 3 * BLK
                        nc.gpsimd.dma_gather(
                            gat[:, 0, k0:k0 + 3].rearrange(
                                "p k x r c -> p k (x r c)"),
                            xt_view,
                            idx_t[:, k0 * (BLK // 16):(k0 + 3) * (BLK // 16)],
                            nch, nch, 1024, elem_step=512,
                            single_packet=False,
                        )
                    nido2 = NIDX // GRP
                    nc.gpsimd.dma_gather(
                        gat[:, 1].rearrange("p k x r c -> p k (x r c)"),
                        xt_view,
                        idx_t[:, (nido2 // 16):],
                        nido2, nido2, 1024, elem_step=512,
                        single_packet=False,
                    )
                elif g <= 4:
                    # per-block gathers during pipeline fill: earlier data
                    # arrival for the first consumed blocks
                    nido2 = NIDX // GRP
                    for blk in range(GRP):
                        nc.gpsimd.dma_gather(
                            gat[:, blk].rearrange("p k x r c -> p k (x r c)"),
                            xt_view,
                            idx_t[:, blk * (nido2 // 16):(blk + 1) * (nido2 // 16)],
                            nido2, nido2, 1024, elem_step=512,
                            single_packet=False,
                        )
                else:
                    nc.gpsimd.dma_gather(
                        gat.rearrange("p a k x r c -> p (a k) (x r c)"),
                        xt_view, idx_t, NIDX, NIDX, 1024, elem_step=512,
                        single_packet=False,
                    )

                for blk in range(GRP):
                    nblk = g * GRP + blk
                    s1_all = kpool.tile([128, 9, 256], f16, tag="s1a", bufs=2)
                    s2_all = kpool.tile([128, 9, 256], f16, tag="s2a", bufs=2)
                    for k in range(9):
                        # 2x2 patch corners, each contiguous 256:
                        # A=(x0,y0) B=(x0,y1) C=(x1,y0) D=(x1,y1)
                        A = gat[:, blk, k, 0, 0, :]
                        Bc = gat[:, blk, k, 0, 1, :]
                        Cc = gat[:, blk, k, 1, 0, :]
                        D = gat[:, blk, k, 1, 1, :]
                        # host-precomputed corner products:
                        # w4f (f32, ACT): wa=(1-fx)(1-fy), wc=fx(1-fy)
                        # w4  (f16, DVE): wb=(1-fx)fy,    wd=fx*fy
                        wa = w4f_t[:, blk, k:k + 1]
                        wc = w4f_t[:, blk, 9 + k:10 + k]
                        wb = w4_t[:, blk, k:k + 1]
                        wd = w4_t[:, blk, 9 + k:10 + k]
                        # two independent ACT->DVE chains per k
                        a_t = kpool.tile([128, 256], f16, tag="ta", bufs=8)
                        nc.scalar.activation(a_t, A, Act.Copy, bias=0.0, scale=wa)
                        c_t = kpool.tile([128, 256], f16, tag="tc", bufs=8)
                        nc.scalar.activation(c_t, Cc, Act.Copy, bias=0.0, scale=wc)
                        nc.vector.scalar_tensor_tensor(
                            s1_all[:, k, :], Bc, wb, a_t, Alu.mult, Alu.add
                        )
                        nc.vector.scalar_tensor_tensor(
                            s2_all[:, k, :], D, wd, c_t, Alu.mult, Alu.add
                        )
                    # one wide add (2304 elems), then 9-way max tree on DVE
                    nc.vector.tensor_tensor(s1_all, s1_all, s2_all, Alu.add)
                    nc.vector.tensor_tensor(
                        s1_all[:, 0:4, :], s1_all[:, 0:4, :], s1_all[:, 4:8, :],
                        Alu.max,
                    )
                    nc.vector.tensor_tensor(
                        s1_all[:, 0:2, :], s1_all[:, 0:2, :], s1_all[:, 2:4, :],
                        Alu.max,
                    )
                    nc.vector.tensor_tensor(
                        s1_all[:, 0, :], s1_all[:, 0, :], s1_all[:, 1, :], Alu.max
                    )
                    q_t = work.tile([128, 256], f16, tag="q")
                    nc.vector.tensor_tensor(
                        q_t, s1_all[:, 0, :], s1_all[:, 8, :], Alu.max
                    )

                    # transpose q -> qT (c-major) via PE
                    qt_ps = pspool.tile([128, 2, 128], f16, tag="qt")
                    for t in range(2):
                        nc.tensor.transpose(
                            qt_ps[:, t, :], q_t[:, t * 128:(t + 1) * 128], ident
                        )
                    qt_sb = work.tile([128, 2, 128], f16, tag="qt_sb")
                    if nblk % 2 == 0:
                        nc.vector.tensor_copy(qt_sb, qt_ps)
                    else:
                        nc.scalar.copy(qt_sb, qt_ps)

                    # Q = qT^T @ w0t  -> [128 pos, 257] (col 256 = sum_o Q)
                    Q_ps = pspool.tile([128, 257], f32, tag="Q", bufs=3)
                    for t in range(2):
                        nc.tensor.matmul(
                            Q_ps, qt_sb[:, t, :], w0t_sb[:, t, :],
                            start=(t == 0), stop=(t == 1),
                        )
                    K_ps = pspool.tile([128, 257], f32, tag="K", bufs=3)
                    for t in range(2):
                        nc.tensor.matmul(
                            K_ps, xk_t[:, t, blk * BLK:(blk + 1) * BLK],
                            w1t_sb[:, t, :], start=(t == 0), stop=(t == 1),
                        )
                    if has_bias:
                        nc.vector.tensor_tensor(Q_ps, Q_ps, qb_sb, Alu.add)
                        nc.vector.tensor_tensor(K_ps, K_ps, kb_sb, Alu.add)

                    # epilogue reductions
                    col = slice(nblk, nblk + 1)
                    act_scr = work.tile([128, 256], f16, tag="act_scr")
                    nc.scalar.activation(
                        act_scr, Q_ps[:, 0:256], Act.Square,
                        accum_out=sqs[:, col],
                    )
                    K_sb = work.tile([128, 256], f16, tag="K_sb")
                    nc.scalar.copy(K_sb, K_ps[:, 0:256])
                    nc.scalar.activation(
                        act_scr, K_ps[:, 0:256], Act.Square, accum_out=sks[:, col],
                    )
                    dve_scr = work.tile([128, 256], f16, tag="dve_scr")
                    nc.vector.scalar_tensor_tensor(
                        dve_scr, Q_ps[:, 0:256], 0.0, K_sb, Alu.bypass, Alu.mult,
                        accum_out=sqks[:, col],
                    )
                    nc.vector.tensor_copy(sQs[:, col], Q_ps[:, 256:257])
                    nc.vector.tensor_copy(sKs[:, col], K_ps[:, 256:257])

            # final combine over [128, NBLK]
            tmp = consts.tile([128, nblk_t], f32, tag="tmp")
            num = consts.tile([128, nblk_t], f32, tag="num")
            dq = consts.tile([128, nblk_t], f32, tag="dq")
            dk = consts.tile([128, nblk_t], f32, tag="dk")
            out_t = consts.tile([128, nblk_t], f32, tag="out")
            inv_c = -1.0 / C
            # num = sqk - sQ*sK/C
            nc.vector.tensor_tensor(tmp, sQs, sKs, Alu.mult)
            nc.vector.scalar_tensor_tensor(num, tmp, inv_c, sqks, Alu.mult, Alu.add)
            # dq = sq - sQ^2/C + eps
            nc.vector.tensor_tensor(tmp, sQs, sQs, Alu.mult)
            nc.vector.scalar_tensor_tensor(dq, tmp, inv_c, sqs, Alu.mult, Alu.add)
            nc.vector.tensor_scalar(dq, dq, EPS, None, Alu.add)
            nc.vector.tensor_tensor(tmp, sKs, sKs, Alu.mult)
            nc.vector.scalar_tensor_tensor(dk, tmp, inv_c, sks, Alu.mult, Alu.add)
            nc.vector.tensor_scalar(dk, dk, EPS, None, Alu.add)
            # out = num / sqrt(dq*dk)
            nc.vector.tensor_tensor(tmp, dq, dk, Alu.mult)
            nc.scalar.activation(tmp, tmp, Act.Sqrt)
            nc.vector.reciprocal(tmp, tmp)
            nc.vector.tensor_tensor(out_t, num, tmp, Alu.mult)
            nc.sync.dma_start(out=o[:, :], in_=out_t)

    nc.compile()
    return nc


def _get_nc(has_bias: bool):
    if has_bias not in _NC_CACHE:
        _NC_CACHE[has_bias] = _build_nc(has_bias)
    return _NC_CACHE[has_bias]


def _build_pair(x_b):
    """Row-pair channel-last layout: A-pairs (even start) then B-pairs (odd),
    with one trailing 512-elem pad element."""
    xp = np.zeros((Hp, Wp, C), np.float16)
    xp[PAD:PAD + H, PAD:PAD + W, :] = x_b.transpose(1, 2, 0)
    pa = xp.reshape(Hp // 2, 2, Wp, C).transpose(0, 2, 1, 3)  # [p, x, r, c]
    pb = xp[1:Hp - 1].reshape(Hp // 2 - 1, 2, Wp, C).transpose(0, 2, 1, 3)
    flat = np.empty(((NPT + 1) * 512,), np.float16)
    flat[:NPA * 512] = pa.reshape(-1)
    flat[NPA * 512:NPT * 512] = pb.reshape(-1)
    flat[NPT * 512:] = 0
    return flat


def _prep_core(x_b, off_b, h0):
    """Host-side shard prep for one core: indices, weights, fp16 layouts."""
    ys, xs = np.meshgrid(
        np.arange(h0, h0 + ROWS), np.arange(W), indexing="ij"
    )
    ys = ys.reshape(-1).astype(np.float32)
    xs = xs.reshape(-1).astype(np.float32)

    idx_all = np.empty((N, 9), np.int32)
    fy_all = np.empty((N, 9), np.float32)
    fx_all = np.empty((N, 9), np.float32)
    for k in range(9):
        kh, kw = k // 3 - 1, k % 3 - 1
        iy = ys.astype(np.int32)
        ix = xs.astype(np.int32)
        py = ys + kh + off_b[2 * k, iy, ix]
        px = xs + kw + off_b[2 * k + 1, iy, ix]
        y0 = np.clip(np.floor(py).astype(np.int32), -PAD, H + PAD - 2)
        x0 = np.clip(np.floor(px).astype(np.int32), -PAD, W + PAD - 2)
        fy_all[:, k] = py - y0
        fx_all[:, k] = px - x0
        yp = y0 + PAD  # [0, 142]
        xpp = x0 + PAD
        even = (yp % 2) == 0
        idx_all[:, k] = np.where(
            even, (yp // 2) * Wp + xpp, NPA + (yp // 2) * Wp + xpp
        )

    # idx tensor [NG, 128, NIDX//16]: slot m = j*128 + p, j = blk*9 + k
    idx_np = np.empty((NG, 128, NIDX // 16), np.int16)
    for g in range(NG):
        slots = np.empty((GRP * 9, BLK), np.int32)
        for blk in range(GRP):
            base = (g * GRP + blk) * BLK
            for k in range(9):
                slots[blk * 9 + k, :] = idx_all[base:base + BLK, k]
        wrapped = slots.reshape(-1).reshape(NIDX // 16, 16).T  # [16, cols]
        idx_np[g] = np.tile(wrapped, (8, 1)).astype(np.int16)

    # corner-product weights: fp16 [NG, 128, GRP, 18] (wb, wd) for DVE,
    # fp32 [.., 18] (wa, wc) for ACT scales
    w4_np = np.empty((NG, 128, GRP, 18), np.float16)
    w4f_np = np.empty((NG, 128, GRP, 18), np.float32)
    fy = fy_all.reshape(NBLK, BLK, 9)
    fx = fx_all.reshape(NBLK, BLK, 9)
    for g in range(NG):
        for blk in range(GRP):
            nb = g * GRP + blk
            w4_np[g, :, blk, 0:9] = (1.0 - fx[nb]) * fy[nb]
            w4_np[g, :, blk, 9:18] = fx[nb] * fy[nb]
            w4f_np[g, :, blk, 0:9] = (1.0 - fx[nb]) * (1.0 - fy[nb])
            w4f_np[g, :, blk, 9:18] = fx[nb] * (1.0 - fy[nb])

    xk_np = np.ascontiguousarray(
        x_b.reshape(2, 128, H, W)[:, :, h0:h0 + ROWS, :].reshape(2, 128, N)
    ).astype(np.float16)
    return idx_np, w4_np, w4f_np, xk_np


def _build_in_maps(x, offset, w0, b0, w1, b1, has_bias):
    w0t_np = np.concatenate([w0.T, w0.sum(0)[:, None]], 1).astype(np.float16)
    w1t_np = np.concatenate([w1.T, w1.sum(0)[:, None]], 1).astype(np.float16)
    w0t_np = np.ascontiguousarray(w0t_np.reshape(2, 128, 257))
    w1t_np = np.ascontiguousarray(w1t_np.reshape(2, 128, 257))

    in_maps = []
    xt_cache = {}
    for core in range(8):
        b, half = core // 2, core % 2
        h0 = ROWS * half
        if b not in xt_cache:
            xt_cache[b] = _build_pair(x[b])
        idx_np, w4_np, w4f_np, xk_np = _prep_core(x[b], offset[b], h0)
        m = {
            "idmat": np.eye(128, dtype=np.float16),
            "xt": xt_cache[b],
            "xk": xk_np,
            "idx": idx_np,
            "w4": w4_np,
            "w4f": w4f_np,
            "w0t": w0t_np,
            "w1t": w1t_np,
        }
        if has_bias:
            qb_np = np.concatenate([b0, [b0.sum()]]).astype(np.float32)
            kb_np = np.concatenate([b1, [b1.sum()]]).astype(np.float32)
            m["qb"] = np.tile(qb_np[None, :], (128, 1))
            m["kb"] = np.tile(kb_np[None, :], (128, 1))
        in_maps.append(m)
    return in_maps


def kernel(x, offset, w0, b0, w1, b1):
    from concourse.bass_utils import run_bass_kernel_spmd

    x = np.asarray(x, np.float32)
    offset = np.asarray(offset, np.float32)
    w0 = np.asarray(w0, np.float32)
    w1 = np.asarray(w1, np.float32)
    b0 = np.asarray(b0, np.float32)
    b1 = np.asarray(b1, np.float32)

    has_bias = bool(np.any(b0)) or bool(np.any(b1))
    nc = _get_nc(has_bias)
    in_maps = _build_in_maps(x, offset, w0, b0, w1, b1, has_bias)

    res = run_bass_kernel_spmd(nc, in_maps, core_ids=list(range(8)))

    out = np.empty((B, 1, H, W), np.float32)
    for core in range(8):
        b, half = core // 2, core % 2
        h0 = ROWS * half
        o = res.results[core]["o"]  # [128 pos(x), 64 rows]
        out[b, 0, h0:h0 + ROWS, :] = o.T
    return out
